# revision 21
# baseline (speedup 1.0000x reference)
"""E3CoordLayer GNN message-passing kernel for 8 Trainium2 NeuronCores.

Strategy (edge-parallel, row-range sharded, v2):
  - Sort edges by row; core c owns rows [c*6250, (c+1)*6250). Rows grouped
    into NB blocks of BLKR=112 rows; within a block, edges split into 2 runs
    by col range (col < 25000 -> lo table, else hi table) so the h gather
    uses 256B single-row descriptors with int16 indices.
  - Each run is padded to RUNW = TP*128 slots (TP = global max, uniform for
    SPMD). Gathers use single_packet=False so descriptors drain across all
    16 SDMA engines instead of one engine per call.
  - h[row] is never gathered: q = h @ W1a computed per node block on device;
    the per-edge expansion q[row_e] uses a host-shipped fp8 one-hot
    M[rel, slot] as the matmul rhs (no on-device M build).
  - The aggregation one-hot oh[slot%128, (t, rel)] is also host-shipped fp8.
  - MLP runs feature-major: z1[h1,e] accumulates W1b^T hcol + W1c'^T ea
    (b1 folded via a ones-row in eaT) + q^T M; silu; z2 = W2^T z1sb; silu
    with b2 bias; z3 per tile via lhsT=z2-tile, rhs=w3 -> z3p[p, r*TP+t];
    tanh once per block; cdt = cdw * sc; agg[3, rel] += cdt^T @ oh in psum.
  - PSUM: zp ring (z1p/z2p [128, RUNW] f32) x2 bufs, z3p [128, 2TP] x2,
    aggp [3, BLKR] x2 -> 8 banks when TP<=8.
  - Gathers keep the xbar-flush guard: z1B mms of run r-2 wait on the
    gather instruction of run r (same-queue reuse distance).
  - Output: per-block (agg + x)*flags -> outT [3, NB*BLKR]; concat cores,
    transpose, trim to [50000, 3].
"""
import sys
import os

sys.path.insert(0, "/opt/trn_rl_repo")

import numpy as np
import ml_dtypes

N_NODES = 50000
N_EDGES = 800000
HIDDEN = 128
EDGE_DIM = 16
COORDS_RANGE = 15.0
NCORES = 8
P = 128
NPC = N_NODES // NCORES          # 6250 nodes per core
BLKR = 112                       # rows per node block
NB = (NPC + BLKR - 1) // BLKR    # 56 blocks per core
NPAD = NB * BLKR                 # 6272 padded nodes per core
C0 = 25000                       # gather table split (int16 idx range)
RCH = 4                          # runs per input chunk (even: 2 blocks)

_BF16 = ml_dtypes.bfloat16
_FP8 = ml_dtypes.float8_e4m3
SINGLE_PACKET = os.environ.get("SP", "1") == "1"
GUARD_DIST = int(os.environ.get("GUARD_DIST", "8"))


def _wrap_idx(idx_call):
    """int16 index list [NI] -> [128, NI//16] (16-part wrap, replicated 8x)."""
    ni = idx_call.shape[0]
    w = idx_call.reshape(ni // 16, 16).T  # [16, NI//16]
    return np.tile(w, (8, 1))             # [128, NI//16]


def _call_sizes(RUNW):
    """Split RUNW into gather-call sizes: multiples of 128, starts at 512
    multiples (so z1 psum chunks never straddle a call), each <= 896."""
    k, rem = RUNW // 512, RUNW % 512
    if k == 0:
        return [RUNW]
    if rem and 512 + rem <= 896:
        return [512] * (k - 1) + [512 + rem]
    return [512] * k + ([rem] if rem else [])


def _build_nc(TP, call_nis):
    import concourse.bass as bass
    import concourse.mybir as mybir
    import concourse.tile as tile
    from concourse import bacc
    from concourse import library_config

    dt = mybir.dt
    RUNW = TP * P                    # edge slots per run
    NRUNS = NB * 2
    S = NRUNS * RUNW                 # edge slots per core
    OHW = NRUNS * TP * BLKR          # oh dram cols
    NCH = (NRUNS + RCH - 1) // RCH   # input chunks
    ED1 = EDGE_DIM + 1

    nc = bacc.Bacc("TRN2", target_bir_lowering=False, debug=False,
                   num_devices=NCORES, num_swdge_queues=4,
                   dynamic_dma_scratch_size=65536)

    hlo = nc.dram_tensor("hlo", [C0 + P, HIDDEN], dt.bfloat16, kind="ExternalInput")
    hhi = nc.dram_tensor("hhi", [N_NODES - C0 + P, HIDDEN], dt.bfloat16, kind="ExternalInput")
    idxw = nc.dram_tensor("idxw", [P, S // 16], dt.int16, kind="ExternalInput")
    Mh = nc.dram_tensor("Mh", [BLKR, S], dt.float8e4, kind="ExternalInput")
    ohh = nc.dram_tensor("ohh", [P, OHW], dt.float8e4, kind="ExternalInput")
    eaT = nc.dram_tensor("eaT", [ED1, S], dt.bfloat16, kind="ExternalInput")
    cdw = nc.dram_tensor("cdw", [P, NRUNS * TP * 3], dt.bfloat16, kind="ExternalInput")
    hTs = nc.dram_tensor("hTs", [P, NPAD], dt.bfloat16, kind="ExternalInput")
    xT3 = nc.dram_tensor("xT3", [3, NPAD], dt.float32, kind="ExternalInput")
    flg3 = nc.dram_tensor("flg3", [3, NPAD], dt.float32, kind="ExternalInput")
    w1a = nc.dram_tensor("w1a", [HIDDEN, HIDDEN], dt.bfloat16, kind="ExternalInput")
    w1b = nc.dram_tensor("w1b", [HIDDEN, HIDDEN], dt.bfloat16, kind="ExternalInput")
    w1c = nc.dram_tensor("w1c", [ED1, HIDDEN], dt.bfloat16, kind="ExternalInput")
    w2 = nc.dram_tensor("w2", [HIDDEN, HIDDEN], dt.bfloat16, kind="ExternalInput")
    w3 = nc.dram_tensor("w3", [HIDDEN, 1], dt.bfloat16, kind="ExternalInput")
    b2 = nc.dram_tensor("b2", [HIDDEN, 1], dt.float32, kind="ExternalInput")
    outT = nc.dram_tensor("outT", [3, NPAD], dt.float32, kind="ExternalOutput")

    AF = mybir.ActivationFunctionType
    ALU = mybir.AluOpType

    # PSUM: 8 banks. z1p/z2p ring wants 3 bufs (so next-run z1 matmuls can
    # start while this run's silus drain) + 1 bank each for z3p/aggp.
    zp_banks = -(-RUNW * 4 // 2048)
    zp_bufs = int(os.environ.get("ZPB", "0")) or (2 if zp_banks <= 2 else 1)
    small_bufs = int(os.environ.get("SMB", "2"))

    with tile.TileContext(nc) as tc:
        nc.gpsimd.load_library(library_config.mlp)
        tc.strict_bb_all_engine_barrier()
        with (
            tc.tile_pool(name="const", bufs=1) as cp,
            tc.tile_pool(name="gath", bufs=GUARD_DIST + 4) as gp,
            tc.tile_pool(name="chunk", bufs=2) as chp,
            tc.tile_pool(name="work", bufs=2) as wp,
            tc.tile_pool(name="small", bufs=2) as scp,
            tc.tile_pool(name="zp", bufs=zp_bufs, space="PSUM") as zp,
            tc.tile_pool(name="zq", bufs=small_bufs, space="PSUM") as zq,
            tc.tile_pool(name="pagg", bufs=small_bufs, space="PSUM") as pa,
        ):
            # ---- resident constants
            w1a_sb = cp.tile([HIDDEN, HIDDEN], dt.bfloat16)
            nc.sync.dma_start(out=w1a_sb[:], in_=w1a[:])
            w1b_sb = cp.tile([HIDDEN, HIDDEN], dt.bfloat16)
            nc.sync.dma_start(out=w1b_sb[:], in_=w1b[:])
            w1c_sb = cp.tile([ED1, HIDDEN], dt.bfloat16)
            nc.sync.dma_start(out=w1c_sb[:], in_=w1c[:])
            w2_sb = cp.tile([HIDDEN, HIDDEN], dt.bfloat16)
            nc.sync.dma_start(out=w2_sb[:], in_=w2[:])
            w3_sb = cp.tile([HIDDEN, 1], dt.bfloat16)
            nc.sync.dma_start(out=w3_sb[:], in_=w3[:])
            b2_sb = cp.tile([HIDDEN, 1], dt.float32)
            nc.sync.dma_start(out=b2_sb[:], in_=b2[:])
            idx_sb = cp.tile([P, S // 16], dt.int16)
            nc.sync.dma_start(out=idx_sb[:], in_=idxw[:])
            cdw_sb = cp.tile([P, NB, 2 * TP, 3], dt.bfloat16)
            nc.sync.dma_start(
                out=cdw_sb[:],
                in_=cdw[:].rearrange("p (b t c) -> p b t c", b=NB, t=2 * TP))
            hTs_sb = cp.tile([P, NPAD], dt.bfloat16)
            nc.sync.dma_start(out=hTs_sb[:], in_=hTs[:])
            x_sb = cp.tile([3, NPAD], dt.float32)
            nc.sync.dma_start(out=x_sb[:], in_=xT3[:])
            f_sb = cp.tile([3, NPAD], dt.float32)
            nc.sync.dma_start(out=f_sb[:], in_=flg3[:])

            # ---- chunked inputs (M, oh, ea) with 1-chunk lookahead
            chunks = {}

            def fetch_chunk(k):
                if k >= NCH or k in chunks:
                    return
                mch = chp.tile([BLKR, RCH * RUNW], dt.float8e4, tag="M")
                nc.sync.dma_start(out=mch[:], in_=Mh[:, k * RCH * RUNW:(k + 1) * RCH * RUNW])
                ohch = chp.tile([P, RCH * TP * BLKR], dt.float8e4, tag="oh")
                nc.sync.dma_start(
                    out=ohch[:],
                    in_=ohh[:, k * RCH * TP * BLKR:(k + 1) * RCH * TP * BLKR])
                each = chp.tile([ED1, RCH * RUNW], dt.bfloat16, tag="ea")
                nc.sync.dma_start(out=each[:], in_=eaT[:, k * RCH * RUNW:(k + 1) * RCH * RUNW])
                chunks[k] = (mch, ohch, each)

            # issue the heavy startup loads before the barrier so the first
            # transposed gathers start on a quiet DMA system
            fetch_chunk(0)
            fetch_chunk(1)

            # ---- q = h @ W1a per node block, node-major [rel, feat]
            q_sb = cp.tile([BLKR, NB, HIDDEN], dt.bfloat16)
            for b in range(NB):
                qp = zp.tile([BLKR, HIDDEN], dt.float32, tag="zp")
                nc.tensor.matmul(qp[:], lhsT=hTs_sb[:, b * BLKR:(b + 1) * BLKR],
                                 rhs=w1a_sb[:], start=True, stop=True)
                nc.vector.tensor_copy(out=q_sb[:, b, :], in_=qp[:])
            tc.strict_bb_all_engine_barrier()

            # ---- main loop
            from concourse.bass import _add_dep_helper
            z1b_by_run = {}
            call_off = [0]
            for ni in call_nis:
                call_off.append(call_off[-1] + ni)
            gcall = 0
            for b in range(NB):
                z3p = zq.tile([P, 2 * TP], dt.float32, tag="z3")
                for r in range(2):
                    run = b * 2 + r
                    k = run // RCH
                    if run % RCH == 0:
                        fetch_chunk(k + 1)
                    mch, ohch, each = chunks[k]
                    roff = (run - k * RCH) * RUNW          # run offset in chunk
                    e0 = run * RUNW                        # first slot of run
                    htab = hlo if r == 0 else hhi

                    # col gathers for the run
                    hcs = []
                    for ci, ni in enumerate(call_nis):
                        hc = gp.tile([P, 1, ni], dt.bfloat16, tag=f"hc{ci}")
                        gi = nc.gpsimd.dma_gather(
                            hc[:], htab[:],
                            idx_sb[:, (e0 + call_off[ci]) // 16:(e0 + call_off[ci + 1]) // 16],
                            ni, ni, HIDDEN, transpose=True,
                            queue_num=gcall % 4, single_packet=SINGLE_PACKET,
                        )
                        gcall += 1
                        # xbar-flush guard: consumers of the gather issued
                        # GUARD_DIST runs earlier wait until this gather
                        # retired on Q7. Same-queue flow control is 2-deep,
                        # so retire(r) proves completion(r-2) fired; dist 4
                        # gives the transposed writes two generations to land.
                        for prev in z1b_by_run.get(run - GUARD_DIST, ()):
                            _add_dep_helper(prev, gi.ins,
                                            reason="gather xbar-flush guard")
                        hcs.append(hc)

                    z1p = zp.tile([P, RUNW], dt.float32, tag="zp")
                    # grouped by weight so LDWEIGHTS is paid once per weight,
                    # not once per (weight, chunk); z1B runs per gather call
                    z1b_list = []
                    for ci, ni in enumerate(call_nis):
                        mm = nc.tensor.matmul(
                            z1p[:, call_off[ci]:call_off[ci + 1]], lhsT=w1b_sb[:],
                            rhs=hcs[ci][:, 0, :],
                            start=True, stop=False)
                        z1b_list.append(mm.ins)
                    for c0 in range(0, RUNW, 512):
                        cw = min(512, RUNW - c0)
                        nc.tensor.matmul(
                            z1p[:, c0:c0 + cw], lhsT=w1c_sb[:],
                            rhs=each[:, roff + c0:roff + c0 + cw],
                            start=False, stop=False)
                    for c0 in range(0, RUNW, 512):
                        cw = min(512, RUNW - c0)
                        nc.tensor.matmul(
                            z1p[:, c0:c0 + cw], lhsT=q_sb[:, b, :],
                            rhs=mch[:, roff + c0:roff + c0 + cw],
                            start=False, stop=True)
                    z1b_by_run[run] = z1b_list

                    z1sb = wp.tile([P, RUNW], dt.bfloat16, tag="z1")
                    nc.scalar.activation(out=z1sb[:], in_=z1p[:], func=AF.Silu)
                    z2p = zp.tile([P, RUNW], dt.float32, tag="zp")
                    for c0 in range(0, RUNW, 512):
                        cw = min(512, RUNW - c0)
                        nc.tensor.matmul(z2p[:, c0:c0 + cw], lhsT=w2_sb[:],
                                         rhs=z1sb[:, c0:c0 + cw], start=True, stop=True)
                    z2sb = wp.tile([P, RUNW], dt.bfloat16, tag="z2")
                    nc.scalar.activation(out=z2sb[:], in_=z2p[:], func=AF.Silu,
                                         bias=b2_sb[:])
                    for t in range(TP):
                        el = t * P
                        nc.tensor.matmul(z3p[:, r * TP + t:r * TP + t + 1],
                                         lhsT=z2sb[:, el:el + P], rhs=w3_sb[:],
                                         start=True, stop=True)

                # ---- block epilogue (after both runs)
                sc = scp.tile([P, 2 * TP], dt.bfloat16, tag="sc")
                nc.scalar.activation(out=sc[:], in_=z3p[:], func=AF.Tanh)
                cdt = scp.tile([P, 2 * TP, 3], dt.bfloat16, tag="cdt")
                nc.vector.tensor_tensor(
                    out=cdt[:], in0=cdw_sb[:, b, :, :],
                    in1=sc[:].to_broadcast([P, 2 * TP, 3]), op=ALU.mult)
                aggp = pa.tile([3, BLKR], dt.float32, tag="agg")
                kb = (2 * b) // RCH
                ohc = chunks[kb][1]
                ooff = (2 * b - kb * RCH) * TP * BLKR
                for t in range(2 * TP):
                    nc.tensor.matmul(
                        aggp[:], lhsT=cdt[:, t, :],
                        rhs=ohc[:, ooff + t * BLKR:ooff + (t + 1) * BLKR],
                        start=(t == 0), stop=(t == 2 * TP - 1))
                osb = scp.tile([3, BLKR], dt.float32, tag="osb")
                nc.vector.tensor_tensor(out=osb[:], in0=aggp[:],
                                        in1=x_sb[:, b * BLKR:(b + 1) * BLKR], op=ALU.add)
                nc.vector.tensor_tensor(out=osb[:], in0=osb[:],
                                        in1=f_sb[:, b * BLKR:(b + 1) * BLKR], op=ALU.mult)
                nc.sync.dma_start(out=outT[:, b * BLKR:(b + 1) * BLKR], in_=osb[:])
                # free chunks fully consumed (keep dict small)
                done = (2 * b + 2) // RCH - 1
                chunks.pop(done - 1, None)
    nc.compile()
    return nc


def _host_prep(h, x, edge_index, edge_attr, coord_diff, flags):
    """Sort/group/pad edges; build per-core input maps.
    Returns (in_maps, TP, call_nis)."""
    row = np.asarray(edge_index[0], dtype=np.int64)
    col = np.asarray(edge_index[1], dtype=np.int64)
    E = row.shape[0]

    core = row // NPC
    rl = row % NPC
    blk = rl // BLKR                        # 0..NB-1
    rel = (rl - blk * BLKR).astype(np.int16)  # 0..BLKR-1
    half = (col >= C0).astype(np.int64)
    key = (core * NB + blk) * 2 + half
    order = np.argsort(key, kind="stable")
    ksort = key[order]
    ngroups = NCORES * NB * 2
    counts = np.bincount(ksort, minlength=ngroups)
    TP = max(int((counts.max() + P - 1) // P), 1)
    RUNW = TP * P
    NRUNS = NB * 2
    S = NRUNS * RUNW
    OHW = NRUNS * TP * BLKR
    call_nis = _call_sizes(RUNW)

    gstart = np.zeros(ngroups + 1, dtype=np.int64)
    gstart[1:] = np.cumsum(counts)
    within = np.arange(E, dtype=np.int64) - gstart[ksort]
    glocal = ksort % NRUNS
    slot = glocal * RUNW + within            # slot on the core
    ecore = ksort // NRUNS

    h_bf = np.ascontiguousarray(np.asarray(h, np.float32).astype(_BF16))
    hlo = np.zeros((C0 + P, HIDDEN), dtype=_BF16)
    hlo[:C0] = h_bf[:C0]
    hhi = np.zeros((N_NODES - C0 + P, HIDDEN), dtype=_BF16)
    hhi[:N_NODES - C0] = h_bf[C0:]
    hT = np.ascontiguousarray(h_bf.T)        # [128, N]

    ea = np.asarray(edge_attr, np.float32)
    cd15 = (np.asarray(coord_diff, np.float32) * COORDS_RANGE).astype(_BF16)

    in_maps = []
    for c in range(NCORES):
        m = ecore == c
        sl = slot[m]
        eidx = order[m]
        relc = rel[eidx]
        tix = sl // P % TP                    # tile within run
        pix = sl % P                          # partition (edge in tile)
        runc = sl // RUNW                     # run index

        idx = np.zeros(S, dtype=np.int16)
        idx[sl] = (col[eidx] - half[eidx] * C0).astype(np.int16)
        Mm = np.zeros((BLKR, S), dtype=_FP8)
        Mm[relc, sl] = np.float32(1.0)
        oh = np.zeros((P, OHW), dtype=_FP8)
        oh[pix, (runc * TP + tix) * BLKR + relc] = np.float32(1.0)
        eaTc = np.zeros((EDGE_DIM + 1, S), dtype=_BF16)
        eaTc[:EDGE_DIM, sl] = ea[eidx].T.astype(_BF16)
        eaTc[EDGE_DIM, sl] = np.float32(1.0)
        cdwc = np.zeros((P, NRUNS * TP * 3), dtype=_BF16)
        cdwc[pix, (runc * TP + tix) * 3 + 0] = cd15[eidx, 0]
        cdwc[pix, (runc * TP + tix) * 3 + 1] = cd15[eidx, 1]
        cdwc[pix, (runc * TP + tix) * 3 + 2] = cd15[eidx, 2]

        idxw = np.zeros((P, S // 16), dtype=np.int16)
        coff = 0
        for g in range(NRUNS):
            base = g * RUNW
            for ni in call_nis:
                idxw[:, coff:coff + ni // 16] = _wrap_idx(idx[base:base + ni])
                base += ni
                coff += ni // 16

        n0 = c * NPC
        hTs = np.zeros((P, NPAD), dtype=_BF16)
        hTs[:, :NPC] = hT[:, n0:n0 + NPC]
        xT3 = np.zeros((3, NPAD), dtype=np.float32)
        xT3[:, :NPC] = np.asarray(x, np.float32)[n0:n0 + NPC].T
        flg3 = np.zeros((3, NPAD), dtype=np.float32)
        flg3[:, :NPC] = np.broadcast_to(
            np.asarray(flags, np.float32)[n0:n0 + NPC].T, (3, NPC))

        in_maps.append({
            "hlo": hlo, "hhi": hhi, "idxw": idxw, "Mh": Mm, "ohh": oh,
            "eaT": np.ascontiguousarray(eaTc), "cdw": cdwc,
            "hTs": hTs, "xT3": xT3, "flg3": flg3,
        })
    return in_maps, TP, call_nis


def kernel(h, x, edge_index, edge_attr, coord_diff, flags, edge_mask,
           W1, b1, W2, b2, W3):
    from concourse.bass_utils import run_bass_kernel_spmd

    h = np.asarray(h, dtype=np.float32)
    x = np.asarray(x, dtype=np.float32)
    in_maps, TP, call_nis = _host_prep(
        h, x, np.asarray(edge_index), np.asarray(edge_attr),
        np.asarray(coord_diff), np.asarray(flags))

    W1 = np.asarray(W1, dtype=np.float32)
    w1c = np.zeros((EDGE_DIM + 1, HIDDEN), dtype=_BF16)
    w1c[:EDGE_DIM] = W1[2 * HIDDEN:].astype(_BF16)
    w1c[EDGE_DIM] = np.asarray(b1, dtype=np.float32).astype(_BF16)
    wshare = {
        "w1a": np.ascontiguousarray(W1[:HIDDEN].astype(_BF16)),
        "w1b": np.ascontiguousarray(W1[HIDDEN:2 * HIDDEN].astype(_BF16)),
        "w1c": w1c,
        "w2": np.ascontiguousarray(np.asarray(W2, np.float32).astype(_BF16)),
        "w3": np.ascontiguousarray(np.asarray(W3, np.float32).astype(_BF16)),
        "b2": np.asarray(b2, np.float32).reshape(HIDDEN, 1),
    }
    for m in in_maps:
        m.update(wshare)

    nc = _build_nc(TP, call_nis)
    res = run_bass_kernel_spmd(nc, in_maps, core_ids=list(range(NCORES)),
                               trace=os.environ.get("BASS_TRACE") == "1")
    global last_result
    last_result = res
    out = np.empty((N_NODES, 3), dtype=np.float32)
    for c in range(NCORES):
        out[c * NPC:(c + 1) * NPC] = res.results[c]["outT"][:, :NPC].T
    return out


last_result = None
